# revision 10
# baseline (speedup 1.0000x reference)
"""GQA kernel for Trainium2, 8 NeuronCores.

Problem: B=2, T=2048, D=2048, 16 query heads / 2 KV heads, d_head=128, causal.

Sharding: core c -> batch b = c//4, head-quarter q = c%4 (query heads
4q..4q+3, kv head q//2). Each core computes its 4 heads' attention and a
partial output projection (its Wo rows); host sums the 4 partials per batch
and adds bo.

Host marshalling: weights and x are pre-cast to bf16 (same rounding the
kernel would do on-chip) and x is supplied transposed (xT = x[b].T), which
is the layout every projection matmul consumes.

On-core dataflow (bf16 matmuls, fp32 PSUM accum):
  QT  = Wq_h.T @ xT   [d_head, T]        KT likewise, V natural [T, d_head]
  S_T = KT_slice.T @ QT  -> [tk, tq] tiles (causal: skip tk > tq blocks)
  P   = exp(S_T * 1/sqrt(d))             (no max-subtraction; |S*scale| < ~5)
  OT  = V.T-accum over tk;  rsum = ones.T @ P (broadcast row-sum)
  OT_norm = OT * 1/rsum;  partial = OT_h.T @ Wo_h rows, summed over heads
"""

import numpy as np
import ml_dtypes
from contextlib import ExitStack

import concourse.bass as bass
from concourse import bacc
import concourse.mybir as mybir
import concourse.tile as tile
from concourse.bass_utils import run_bass_kernel_spmd

F32 = mybir.dt.float32
BF16 = mybir.dt.bfloat16

D = 2048
T = 2048
DH = 128
B = 2
HPC = 4            # query heads per core
NCORES = 8
SCALE = 1.0 / float(np.sqrt(128.0))
MASK_NEG = -30000.0  # exp(MASK_NEG * SCALE) == 0

_CACHE = {}


def _build_nc():
    nc = bacc.Bacc("TRN2", target_bir_lowering=False, debug=False,
                   num_devices=NCORES)

    xt = nc.dram_tensor("xt", [D, T], BF16, kind="ExternalInput")
    wq = nc.dram_tensor("wq", [D, HPC * DH], BF16, kind="ExternalInput")
    wk = nc.dram_tensor("wk", [D, DH], BF16, kind="ExternalInput")
    wv = nc.dram_tensor("wv", [D, DH], BF16, kind="ExternalInput")
    wo = nc.dram_tensor("wo", [HPC * DH, D], BF16, kind="ExternalInput")
    bqm = nc.dram_tensor("bqm", [DH, HPC], F32, kind="ExternalInput")
    bkm = nc.dram_tensor("bkm", [DH, 1], F32, kind="ExternalInput")
    bvb = nc.dram_tensor("bvb", [DH, DH], F32, kind="ExternalInput")
    part = nc.dram_tensor("part", [T, D], F32, kind="ExternalOutput")

    with ExitStack() as ctx:
        tc = ctx.enter_context(tile.TileContext(nc))
        persist = ctx.enter_context(tc.tile_pool(name="persist", bufs=1))
        work = ctx.enter_context(tc.tile_pool(name="work", bufs=3))
        psum = ctx.enter_context(tc.tile_pool(name="psum", bufs=2, space="PSUM"))

        # ---- constants ----
        ones = persist.tile([128, 128], BF16, tag="ones", name="ones")
        nc.vector.memset(ones, 1.0)

        bq_sb = persist.tile([DH, HPC], F32, tag="bq", name="bq_sb")
        nc.sync.dma_start(out=bq_sb, in_=bqm[:, :])
        bk_sb = persist.tile([DH, 1], F32, tag="bk", name="bk_sb")
        nc.sync.dma_start(out=bk_sb, in_=bkm[:, :])
        bvb_sb = persist.tile([DH, DH], F32, tag="bv", name="bvb_sb")
        nc.sync.dma_start(out=bvb_sb, in_=bvb[:, :])

        # causal masks for the 4 diagonal sub-block offsets: keep where
        # tq_free >= tk_part + 128*r  (within a 512-wide tq tile)
        masks = []
        for r in range(4):
            m = persist.tile([128, 512], BF16, tag=f"mask{r}", name=f"mask{r}")
            nc.gpsimd.memset(m, 0.0)
            nc.gpsimd.affine_select(
                out=m, in_=m,
                compare_op=mybir.AluOpType.is_ge,
                fill=MASK_NEG,
                base=-(128 * r),
                pattern=[[1, 512]],
                channel_multiplier=-1,
            )
            masks.append(m)

        # ---- inputs -> SBUF (already bf16) ----
        xT = []
        wq_sb = []
        wk_sb = []
        wv_sb = []
        for kb in range(16):
            xtt = persist.tile([128, T], BF16, tag=f"xT{kb}", name=f"xT{kb}")
            nc.sync.dma_start(out=xtt, in_=xt[kb * 128:(kb + 1) * 128, :])
            xT.append(xtt)
            wqt = persist.tile([128, 512], BF16, tag=f"wq{kb}", name=f"wq_sb{kb}")
            nc.sync.dma_start(out=wqt, in_=wq[kb * 128:(kb + 1) * 128, :])
            wq_sb.append(wqt)
            wkt = persist.tile([128, 128], BF16, tag=f"wk{kb}", name=f"wk_sb{kb}")
            nc.sync.dma_start(out=wkt, in_=wk[kb * 128:(kb + 1) * 128, :])
            wk_sb.append(wkt)
            wvt = persist.tile([128, 128], BF16, tag=f"wv{kb}", name=f"wv_sb{kb}")
            nc.sync.dma_start(out=wvt, in_=wv[kb * 128:(kb + 1) * 128, :])
            wv_sb.append(wvt)

        wo_sb = []
        for h in range(HPC):
            wot = persist.tile([128, D], BF16, tag=f"wo{h}", name=f"wo_sb{h}")
            nc.sync.dma_start(out=wot, in_=wo[h * 128:(h + 1) * 128, :])
            wo_sb.append(wot)

        # ---- persistent activations ----
        qT = [persist.tile([128, T], BF16, tag=f"qT{h}", name=f"qT{h}")
              for h in range(HPC)]
        kT = persist.tile([128, T], BF16, tag="kT", name="kT")
        v_sb = [persist.tile([128, DH], BF16, tag=f"v{t}", name=f"v{t}")
                for t in range(16)]
        oT = [persist.tile([128, T], BF16, tag=f"oT{h}", name=f"oT{h}")
              for h in range(HPC)]

        # ---- phase B: Q/K/V projections per 512-col t-slice ----
        for ts in range(4):
            sl = slice(ts * 512, (ts + 1) * 512)
            kps = psum.tile([128, 512], F32, tag="acc", bufs=2,
                            name=f"kps{ts}")
            for kb in range(16):
                nc.tensor.matmul(out=kps, lhsT=wk_sb[kb], rhs=xT[kb][:, sl],
                                 start=(kb == 0), stop=(kb == 15))
            nc.scalar.activation(out=kT[:, sl], in_=kps,
                                 func=mybir.ActivationFunctionType.Identity,
                                 bias=bk_sb[:, 0:1], scale=1.0)

            for h in range(HPC):
                qps = psum.tile([128, 512], F32, tag="acc", bufs=2,
                                name=f"qps{ts}_{h}")
                for kb in range(16):
                    nc.tensor.matmul(out=qps,
                                     lhsT=wq_sb[kb][:, h * 128:(h + 1) * 128],
                                     rhs=xT[kb][:, sl],
                                     start=(kb == 0), stop=(kb == 15))
                nc.scalar.activation(out=qT[h][:, sl], in_=qps,
                                     func=mybir.ActivationFunctionType.Identity,
                                     bias=bq_sb[:, h:h + 1], scale=1.0)

            for sub in range(4):
                tt = 4 * ts + sub
                vps = psum.tile([128, DH], F32, tag="rv", bufs=2,
                                name=f"vps{tt}")
                for kb in range(16):
                    nc.tensor.matmul(out=vps,
                                     lhsT=xT[kb][:, tt * 128:(tt + 1) * 128],
                                     rhs=wv_sb[kb],
                                     start=(kb == 0), stop=(kb == 15))
                nc.vector.tensor_add(out=v_sb[tt], in0=vps, in1=bvb_sb)

        # ---- phase C: attention ----
        for h in range(HPC):
            for j in range(4):
                ntk = 4 * (j + 1)
                otps = psum.tile([128, 512], F32, tag="acc", bufs=2,
                                 name=f"otps{h}_{j}")
                rsps = psum.tile([128, 512], F32, tag="rv", bufs=2,
                                 name=f"rsps{h}_{j}")
                for tkb in range(ntk):
                    sps = psum.tile([128, 512], F32, tag="sp", bufs=2,
                                    name=f"sps{h}_{j}_{tkb}")
                    nc.tensor.matmul(out=sps,
                                     lhsT=kT[:, tkb * 128:(tkb + 1) * 128],
                                     rhs=qT[h][:, j * 512:(j + 1) * 512],
                                     start=True, stop=True)
                    if tkb >= 4 * j:
                        nc.vector.tensor_add(out=sps, in0=sps,
                                             in1=masks[tkb - 4 * j])
                    pt = work.tile([128, 512], BF16, tag="pt", bufs=3,
                                   name=f"pt{h}_{j}_{tkb}")
                    nc.scalar.activation(out=pt, in_=sps,
                                         func=mybir.ActivationFunctionType.Exp,
                                         scale=SCALE)
                    nc.tensor.matmul(out=otps, lhsT=v_sb[tkb], rhs=pt,
                                     start=(tkb == 0), stop=(tkb == ntk - 1))
                    nc.tensor.matmul(out=rsps, lhsT=ones, rhs=pt,
                                     start=(tkb == 0), stop=(tkb == ntk - 1))
                rinv = work.tile([128, 512], F32, tag="rinv", bufs=2,
                                 name=f"rinv{h}_{j}")
                nc.vector.reciprocal(rinv, rsps)
                nc.vector.tensor_mul(out=oT[h][:, j * 512:(j + 1) * 512],
                                     in0=otps, in1=rinv)

        # ---- phase D: output projection (partial over this core's heads) ----
        for tt in range(16):
            ostg = work.tile([128, D], F32, tag="ostg", bufs=2,
                             name=f"ostg{tt}")
            for n in range(4):
                ops = psum.tile([128, 512], F32, tag="op", bufs=2,
                                name=f"ops{tt}_{n}")
                for h in range(HPC):
                    nc.tensor.matmul(out=ops,
                                     lhsT=oT[h][:, tt * 128:(tt + 1) * 128],
                                     rhs=wo_sb[h][:, n * 512:(n + 1) * 512],
                                     start=(h == 0), stop=(h == HPC - 1))
                nc.scalar.copy(out=ostg[:, n * 512:(n + 1) * 512], in_=ops)
            nc.sync.dma_start(out=part[tt * 128:(tt + 1) * 128, :], in_=ostg)

    nc.compile()
    return nc


def _get_nc():
    if "nc" not in _CACHE:
        _CACHE["nc"] = _build_nc()
    return _CACHE["nc"]


def _bf16(a):
    return np.ascontiguousarray(a.astype(ml_dtypes.bfloat16))


def kernel(x, Wq, bq, Wk, bk, Wv, bv, Wo, bo, **kw):
    x = np.asarray(x, dtype=np.float32)
    Wq = np.asarray(Wq, dtype=np.float32)
    Wk = np.asarray(Wk, dtype=np.float32)
    Wv = np.asarray(Wv, dtype=np.float32)
    Wo = np.asarray(Wo, dtype=np.float32)
    bq = np.asarray(bq, dtype=np.float32)
    bk = np.asarray(bk, dtype=np.float32)
    bv = np.asarray(bv, dtype=np.float32)
    bo = np.asarray(bo, dtype=np.float32)

    nc = _get_nc()
    xt_b = [_bf16(x[b].T) for b in range(B)]
    in_maps = []
    for c in range(NCORES):
        b = c // 4
        q = c % 4
        hs = q * HPC * DH          # column start in Wq / row start in Wo
        kv = q // 2
        bq_m = np.ascontiguousarray(
            bq[hs:hs + HPC * DH].reshape(HPC, DH).T)          # [128, 4]
        bk_m = np.ascontiguousarray(
            bk[kv * DH:(kv + 1) * DH].reshape(DH, 1))         # [128, 1]
        bv_b = np.ascontiguousarray(
            np.broadcast_to(bv[kv * DH:(kv + 1) * DH], (DH, DH)))  # [128,128]
        in_maps.append({
            "xt": xt_b[b],
            "wq": _bf16(Wq[:, hs:hs + HPC * DH]),
            "wk": _bf16(Wk[:, kv * DH:(kv + 1) * DH]),
            "wv": _bf16(Wv[:, kv * DH:(kv + 1) * DH]),
            "wo": _bf16(Wo[hs:hs + HPC * DH, :]),
            "bqm": bq_m,
            "bkm": bk_m,
            "bvb": bv_b,
        })

    res = run_bass_kernel_spmd(nc, in_maps, list(range(NCORES)),
                               **kw.get("_run_kwargs", {}))
    if kw.get("_return_res"):
        return res
    parts = [res.results[c]["part"] for c in range(NCORES)]
    out = np.empty((B, T, D), dtype=np.float32)
    for b in range(B):
        acc = parts[4 * b].astype(np.float32).copy()
        for q in range(1, 4):
            acc += parts[4 * b + q]
        out[b] = acc + bo[None, :]
    return out


# revision 11
# speedup vs baseline: 1.0353x; 1.0353x over previous
"""GQA kernel for Trainium2, 8 NeuronCores.

Problem: B=2, T=2048, D=2048, 16 query heads / 2 KV heads, d_head=128, causal.

Sharding: core c -> batch b = c//4, head-quarter q = c%4 (query heads
4q..4q+3, kv head q//2). Each core computes its 4 heads' attention and a
partial output projection (its Wo rows); host sums the 4 partials per batch
and adds bo.

Host marshalling: weights and x are pre-cast to bf16 (same rounding the
kernel would do on-chip) and x is supplied transposed (xT = x[b].T), which
is the layout every projection matmul consumes.

On-core dataflow (bf16 matmuls, fp32 PSUM accum), interleaved in 4 rounds
over 512-wide t-slices so PE stays continuously fed:
  round j: project KT/QT/VT for slice j; PE-transpose VT -> V natural;
           attention (h, j) for all 4 heads over tk blocks 0..4j+3
           (S_T tiles [tk,tq], causal skip, exp via ACT, OT/rowsum accum);
           output projection for the 4 t-tiles of slice j.
"""

import numpy as np
import ml_dtypes
from contextlib import ExitStack

import concourse.bass as bass
from concourse import bacc
import concourse.mybir as mybir
import concourse.tile as tile
from concourse.bass_utils import run_bass_kernel_spmd
from concourse.masks import make_identity

F32 = mybir.dt.float32
BF16 = mybir.dt.bfloat16

D = 2048
T = 2048
DH = 128
B = 2
HPC = 4            # query heads per core
NCORES = 8
SCALE = 1.0 / float(np.sqrt(128.0))
MASK_NEG = -30000.0  # exp(MASK_NEG * SCALE) == 0

_CACHE = {}


def _build_nc():
    nc = bacc.Bacc("TRN2", target_bir_lowering=False, debug=False,
                   num_devices=NCORES)

    xt = nc.dram_tensor("xt", [D, T], BF16, kind="ExternalInput")
    wq = nc.dram_tensor("wq", [D, HPC * DH], BF16, kind="ExternalInput")
    wk = nc.dram_tensor("wk", [D, DH], BF16, kind="ExternalInput")
    wv = nc.dram_tensor("wv", [D, DH], BF16, kind="ExternalInput")
    wo = nc.dram_tensor("wo", [HPC * DH, D], BF16, kind="ExternalInput")
    bqm = nc.dram_tensor("bqm", [DH, HPC], F32, kind="ExternalInput")
    bkm = nc.dram_tensor("bkm", [DH, 1], F32, kind="ExternalInput")
    bvm = nc.dram_tensor("bvm", [DH, 1], F32, kind="ExternalInput")
    part = nc.dram_tensor("part", [T, D], F32, kind="ExternalOutput")

    with ExitStack() as ctx:
        tc = ctx.enter_context(tile.TileContext(nc))
        persist = ctx.enter_context(tc.tile_pool(name="persist", bufs=1))
        work = ctx.enter_context(tc.tile_pool(name="work", bufs=3))
        psum = ctx.enter_context(tc.tile_pool(name="psum", bufs=2, space="PSUM"))

        # ---- constants ----
        ones = persist.tile([128, 128], BF16, tag="ones", name="ones")
        nc.vector.memset(ones, 1.0)
        ident = persist.tile([128, 128], BF16, tag="ident", name="ident")
        make_identity(nc, ident)

        bq_sb = persist.tile([DH, HPC], F32, tag="bq", name="bq_sb")
        nc.sync.dma_start(out=bq_sb, in_=bqm[:, :])
        bk_sb = persist.tile([DH, 1], F32, tag="bk", name="bk_sb")
        nc.sync.dma_start(out=bk_sb, in_=bkm[:, :])
        bv_sb = persist.tile([DH, 1], F32, tag="bv", name="bv_sb")
        nc.sync.dma_start(out=bv_sb, in_=bvm[:, :])

        # causal masks for the 4 diagonal sub-block offsets: keep where
        # tq_free >= tk_part + 128*r  (within a 512-wide tq tile)
        masks = []
        for r in range(4):
            m = persist.tile([128, 512], BF16, tag=f"mask{r}", name=f"mask{r}")
            nc.gpsimd.memset(m, 0.0)
            nc.gpsimd.affine_select(
                out=m, in_=m,
                compare_op=mybir.AluOpType.is_ge,
                fill=MASK_NEG,
                base=-(128 * r),
                pattern=[[1, 512]],
                channel_multiplier=-1,
            )
            masks.append(m)

        # ---- inputs -> SBUF (already bf16) ----
        xT = []
        wq_sb = []
        wk_sb = []
        wv_sb = []
        for kb in range(16):
            xtt = persist.tile([128, T], BF16, tag=f"xT{kb}", name=f"xT{kb}")
            nc.sync.dma_start(out=xtt, in_=xt[kb * 128:(kb + 1) * 128, :])
            xT.append(xtt)
            wqt = persist.tile([128, 512], BF16, tag=f"wq{kb}", name=f"wq_sb{kb}")
            nc.sync.dma_start(out=wqt, in_=wq[kb * 128:(kb + 1) * 128, :])
            wq_sb.append(wqt)
            wkt = persist.tile([128, 128], BF16, tag=f"wk{kb}", name=f"wk_sb{kb}")
            nc.sync.dma_start(out=wkt, in_=wk[kb * 128:(kb + 1) * 128, :])
            wk_sb.append(wkt)
            wvt = persist.tile([128, 128], BF16, tag=f"wv{kb}", name=f"wv_sb{kb}")
            nc.sync.dma_start(out=wvt, in_=wv[kb * 128:(kb + 1) * 128, :])
            wv_sb.append(wvt)

        wo_sb = []
        for h in range(HPC):
            wot = persist.tile([128, D], BF16, tag=f"wo{h}", name=f"wo_sb{h}")
            nc.sync.dma_start(out=wot, in_=wo[h * 128:(h + 1) * 128, :])
            wo_sb.append(wot)

        # ---- persistent activations ----
        qT = [persist.tile([128, T], BF16, tag=f"qT{h}", name=f"qT{h}")
              for h in range(HPC)]
        kT = persist.tile([128, T], BF16, tag="kT", name="kT")
        v_sb = [persist.tile([128, DH], BF16, tag=f"v{t}", name=f"v{t}")
                for t in range(16)]
        oT = [persist.tile([128, T], BF16, tag=f"oT{h}", name=f"oT{h}")
              for h in range(HPC)]

        for j in range(4):
            sl = slice(j * 512, (j + 1) * 512)

            # --- projections for t-slice j ---
            kps = psum.tile([128, 512], F32, tag="acc", bufs=2, name=f"kps{j}")
            for kb in range(16):
                nc.tensor.matmul(out=kps, lhsT=wk_sb[kb], rhs=xT[kb][:, sl],
                                 start=(kb == 0), stop=(kb == 15))
            nc.scalar.activation(out=kT[:, sl], in_=kps,
                                 func=mybir.ActivationFunctionType.Identity,
                                 bias=bk_sb[:, 0:1], scale=1.0)

            for h in range(HPC):
                qps = psum.tile([128, 512], F32, tag="acc", bufs=2,
                                name=f"qps{j}_{h}")
                for kb in range(16):
                    nc.tensor.matmul(out=qps,
                                     lhsT=wq_sb[kb][:, h * 128:(h + 1) * 128],
                                     rhs=xT[kb][:, sl],
                                     start=(kb == 0), stop=(kb == 15))
                nc.scalar.activation(out=qT[h][:, sl], in_=qps,
                                     func=mybir.ActivationFunctionType.Identity,
                                     bias=bq_sb[:, h:h + 1], scale=1.0)

            # VT projection for slice j, then PE-transpose to natural V
            vps = psum.tile([128, 512], F32, tag="acc", bufs=2, name=f"vps{j}")
            for kb in range(16):
                nc.tensor.matmul(out=vps, lhsT=wv_sb[kb], rhs=xT[kb][:, sl],
                                 start=(kb == 0), stop=(kb == 15))
            vt_sb = work.tile([128, 512], BF16, tag="vt", bufs=2,
                              name=f"vt{j}")
            nc.scalar.activation(out=vt_sb, in_=vps,
                                 func=mybir.ActivationFunctionType.Identity,
                                 bias=bv_sb[:, 0:1], scale=1.0)
            vtp = psum.tile([128, 512], BF16, tag="op", bufs=2, name=f"vtp{j}")
            for sub in range(4):
                nc.tensor.transpose(vtp[:, sub * 128:(sub + 1) * 128],
                                    vt_sb[:, sub * 128:(sub + 1) * 128],
                                    ident)
            for sub in range(4):
                nc.scalar.copy(out=v_sb[4 * j + sub],
                               in_=vtp[:, sub * 128:(sub + 1) * 128])

            # --- attention for all heads, tq-slice j ---
            ntk = 4 * (j + 1)
            for h in range(HPC):
                otps = psum.tile([128, 512], F32, tag="acc", bufs=2,
                                 name=f"otps{h}_{j}")
                rsps = psum.tile([128, 512], F32, tag="rv", bufs=2,
                                 name=f"rsps{h}_{j}")
                for tkb in range(ntk):
                    sps = psum.tile([128, 512], F32, tag="sp", bufs=2,
                                    name=f"sps{h}_{j}_{tkb}")
                    nc.tensor.matmul(out=sps,
                                     lhsT=kT[:, tkb * 128:(tkb + 1) * 128],
                                     rhs=qT[h][:, sl],
                                     start=True, stop=True)
                    if tkb >= 4 * j:
                        nc.vector.tensor_add(out=sps, in0=sps,
                                             in1=masks[tkb - 4 * j])
                    pt = work.tile([128, 512], BF16, tag="pt", bufs=3,
                                   name=f"pt{h}_{j}_{tkb}")
                    nc.scalar.activation(out=pt, in_=sps,
                                         func=mybir.ActivationFunctionType.Exp,
                                         scale=SCALE)
                    nc.tensor.matmul(out=otps, lhsT=v_sb[tkb], rhs=pt,
                                     start=(tkb == 0), stop=(tkb == ntk - 1))
                    nc.tensor.matmul(out=rsps, lhsT=ones, rhs=pt,
                                     start=(tkb == 0), stop=(tkb == ntk - 1))
                rinv = work.tile([128, 512], F32, tag="rinv", bufs=2,
                                 name=f"rinv{h}_{j}")
                nc.vector.reciprocal_approx_fast(rinv, rsps)
                nc.vector.tensor_mul(out=oT[h][:, sl], in0=otps, in1=rinv)

            # --- output projection for the 4 t-tiles of slice j ---
            for sub in range(4):
                tt = 4 * j + sub
                ostg = work.tile([128, D], F32, tag="ostg", bufs=2,
                                 name=f"ostg{tt}")
                for n in range(4):
                    ops = psum.tile([128, 512], F32, tag="op", bufs=2,
                                    name=f"ops{tt}_{n}")
                    for h in range(HPC):
                        nc.tensor.matmul(
                            out=ops,
                            lhsT=oT[h][:, tt * 128:(tt + 1) * 128],
                            rhs=wo_sb[h][:, n * 512:(n + 1) * 512],
                            start=(h == 0), stop=(h == HPC - 1))
                    nc.scalar.copy(out=ostg[:, n * 512:(n + 1) * 512], in_=ops)
                nc.sync.dma_start(out=part[tt * 128:(tt + 1) * 128, :],
                                  in_=ostg)

    nc.compile()
    return nc


def _get_nc():
    if "nc" not in _CACHE:
        _CACHE["nc"] = _build_nc()
    return _CACHE["nc"]


def _bf16(a):
    return np.ascontiguousarray(a.astype(ml_dtypes.bfloat16))


def kernel(x, Wq, bq, Wk, bk, Wv, bv, Wo, bo, **kw):
    x = np.asarray(x, dtype=np.float32)
    Wq = np.asarray(Wq, dtype=np.float32)
    Wk = np.asarray(Wk, dtype=np.float32)
    Wv = np.asarray(Wv, dtype=np.float32)
    Wo = np.asarray(Wo, dtype=np.float32)
    bq = np.asarray(bq, dtype=np.float32)
    bk = np.asarray(bk, dtype=np.float32)
    bv = np.asarray(bv, dtype=np.float32)
    bo = np.asarray(bo, dtype=np.float32)

    nc = _get_nc()
    xt_b = [_bf16(x[b].T) for b in range(B)]
    in_maps = []
    for c in range(NCORES):
        b = c // 4
        q = c % 4
        hs = q * HPC * DH          # column start in Wq / row start in Wo
        kv = q // 2
        bq_m = np.ascontiguousarray(
            bq[hs:hs + HPC * DH].reshape(HPC, DH).T)          # [128, 4]
        bk_m = np.ascontiguousarray(
            bk[kv * DH:(kv + 1) * DH].reshape(DH, 1))         # [128, 1]
        bv_m = np.ascontiguousarray(
            bv[kv * DH:(kv + 1) * DH].reshape(DH, 1))         # [128, 1]
        in_maps.append({
            "xt": xt_b[b],
            "wq": _bf16(Wq[:, hs:hs + HPC * DH]),
            "wk": _bf16(Wk[:, kv * DH:(kv + 1) * DH]),
            "wv": _bf16(Wv[:, kv * DH:(kv + 1) * DH]),
            "wo": _bf16(Wo[hs:hs + HPC * DH, :]),
            "bqm": bq_m,
            "bkm": bk_m,
            "bvm": bv_m,
        })

    res = run_bass_kernel_spmd(nc, in_maps, list(range(NCORES)),
                               **kw.get("_run_kwargs", {}))
    if kw.get("_return_res"):
        return res
    parts = [res.results[c]["part"] for c in range(NCORES)]
    out = np.empty((B, T, D), dtype=np.float32)
    for b in range(B):
        acc = parts[4 * b].astype(np.float32).copy()
        for q in range(1, 4):
            acc += parts[4 * b + q]
        out[b] = acc + bo[None, :]
    return out


# revision 13
# speedup vs baseline: 1.0651x; 1.0288x over previous
"""GQA kernel for Trainium2, 8 NeuronCores.

Problem: B=2, T=2048, D=2048, 16 query heads / 2 KV heads, d_head=128, causal.

Sharding: core c -> batch b = c//4, head-quarter q = c%4 (query heads
4q..4q+3, kv head q//2). Each core computes its 4 heads' attention and a
partial output projection (its Wo rows); host sums the 4 partials per batch
and adds bo.

Host marshalling: weights and x are pre-cast to bf16 (same rounding the
kernel would do on-chip) and x is supplied transposed (xT = x[b].T), which
is the layout every projection matmul consumes.

On-core dataflow (bf16 matmuls, fp32 PSUM accum), interleaved in 4 rounds
over 512-wide t-slices so PE stays continuously fed:
  round j: project KT/QT/VT for slice j; PE-transpose VT -> V natural;
           attention (h, j) for all 4 heads over tk blocks 0..4j+3
           (S_T tiles [tk,tq]; exp on ACT; causal zeroing of the diagonal
           blocks on GpSimd post-exp; OT accum on PE; row-sum accum on DVE
           with one fp32 ones-matmul per (h,j) for the partition reduction);
           output projection for the 4 t-tiles of slice j.
Engine budget: PE ~matmuls only, ACT ~exp+proj epilogues, DVE ~copies+
row-sum+normalize, GpSimd ~causal masks, 4 DMA queues for input streaming.
"""

import numpy as np
import ml_dtypes
from contextlib import ExitStack

import concourse.bass as bass
from concourse import bacc
import concourse.mybir as mybir
import concourse.tile as tile
from concourse.bass_utils import run_bass_kernel_spmd
from concourse.masks import make_identity

F32 = mybir.dt.float32
BF16 = mybir.dt.bfloat16

D = 2048
T = 2048
DH = 128
B = 2
HPC = 4            # query heads per core
NCORES = 8
SCALE = 1.0 / float(np.sqrt(128.0))

_CACHE = {}


def _build_nc():
    nc = bacc.Bacc("TRN2", target_bir_lowering=False, debug=False,
                   num_devices=NCORES)

    xt = nc.dram_tensor("xt", [D, T], BF16, kind="ExternalInput")
    wq = nc.dram_tensor("wq", [D, HPC * DH], BF16, kind="ExternalInput")
    wk = nc.dram_tensor("wk", [D, DH], BF16, kind="ExternalInput")
    wv = nc.dram_tensor("wv", [D, DH], BF16, kind="ExternalInput")
    wo = nc.dram_tensor("wo", [HPC * DH, D], BF16, kind="ExternalInput")
    bqm = nc.dram_tensor("bqm", [DH, HPC], F32, kind="ExternalInput")
    bkm = nc.dram_tensor("bkm", [DH, 1], F32, kind="ExternalInput")
    bvm = nc.dram_tensor("bvm", [DH, 1], F32, kind="ExternalInput")
    part = nc.dram_tensor("part", [T, D], F32, kind="ExternalOutput")

    with ExitStack() as ctx:
        tc = ctx.enter_context(tile.TileContext(nc))
        persist = ctx.enter_context(tc.tile_pool(name="persist", bufs=1))
        work = ctx.enter_context(tc.tile_pool(name="work", bufs=3))
        psum = ctx.enter_context(tc.tile_pool(name="psum", bufs=2, space="PSUM"))

        # ---- constants ----
        ones32 = persist.tile([128, 128], F32, tag="ones32", name="ones32")
        nc.vector.memset(ones32, 1.0)
        ident = persist.tile([128, 128], BF16, tag="ident", name="ident")
        make_identity(nc, ident)

        bq_sb = persist.tile([DH, HPC], F32, tag="bq", name="bq_sb")
        nc.sync.dma_start(out=bq_sb, in_=bqm[:, :])
        bk_sb = persist.tile([DH, 1], F32, tag="bk", name="bk_sb")
        nc.sync.dma_start(out=bk_sb, in_=bkm[:, :])
        bv_sb = persist.tile([DH, 1], F32, tag="bv", name="bv_sb")
        nc.sync.dma_start(out=bv_sb, in_=bvm[:, :])

        # ---- inputs -> SBUF (already bf16), streamed on 4 DMA queues in
        # consumption order: wk, xT slice 0, wq, wv, xT slices 1-3, wo ----
        queues = [nc.sync, nc.scalar, nc.gpsimd]
        _qi = [0]

        def dma(out, in_):
            queues[_qi[0] % 3].dma_start(out=out, in_=in_)
            _qi[0] += 1

        xT = [persist.tile([128, T], BF16, tag=f"xT{kb}", name=f"xT{kb}")
              for kb in range(16)]
        wq_sb = []
        wk_sb = []
        wv_sb = []
        for kb in range(16):
            wkt = persist.tile([128, 128], BF16, tag=f"wk{kb}", name=f"wk_sb{kb}")
            dma(wkt, wk[kb * 128:(kb + 1) * 128, :])
            wk_sb.append(wkt)
        for kb in range(16):
            dma(xT[kb][:, 0:512], xt[kb * 128:(kb + 1) * 128, 0:512])
        for kb in range(16):
            wqt = persist.tile([128, 512], BF16, tag=f"wq{kb}", name=f"wq_sb{kb}")
            dma(wqt, wq[kb * 128:(kb + 1) * 128, :])
            wq_sb.append(wqt)
        for kb in range(16):
            wvt = persist.tile([128, 128], BF16, tag=f"wv{kb}", name=f"wv_sb{kb}")
            dma(wvt, wv[kb * 128:(kb + 1) * 128, :])
            wv_sb.append(wvt)
        for js in range(1, 4):
            for kb in range(16):
                dma(xT[kb][:, js * 512:(js + 1) * 512],
                    xt[kb * 128:(kb + 1) * 128, js * 512:(js + 1) * 512])
        wo_sb = []
        for h in range(HPC):
            wot = persist.tile([128, D], BF16, tag=f"wo{h}", name=f"wo_sb{h}")
            dma(wot, wo[h * 128:(h + 1) * 128, :])
            wo_sb.append(wot)

        # ---- persistent activations ----
        qT = [persist.tile([128, T], BF16, tag=f"qT{h}", name=f"qT{h}")
              for h in range(HPC)]
        kT = persist.tile([128, T], BF16, tag="kT", name="kT")
        v_sb = [persist.tile([128, DH], BF16, tag=f"v{t}", name=f"v{t}")
                for t in range(16)]
        oT = [persist.tile([128, T], BF16, tag=f"oT{h}", name=f"oT{h}")
              for h in range(HPC)]

        for j in range(4):
            sl = slice(j * 512, (j + 1) * 512)

            # --- projections for t-slice j ---
            kps = psum.tile([128, 512], F32, tag="acc", bufs=3, name=f"kps{j}")
            for kb in range(16):
                nc.tensor.matmul(out=kps, lhsT=wk_sb[kb], rhs=xT[kb][:, sl],
                                 start=(kb == 0), stop=(kb == 15))
            nc.scalar.activation(out=kT[:, sl], in_=kps,
                                 func=mybir.ActivationFunctionType.Identity,
                                 bias=bk_sb[:, 0:1], scale=1.0)

            for h in range(HPC):
                qps = psum.tile([128, 512], F32, tag="acc", bufs=3,
                                name=f"qps{j}_{h}")
                for kb in range(16):
                    nc.tensor.matmul(out=qps,
                                     lhsT=wq_sb[kb][:, h * 128:(h + 1) * 128],
                                     rhs=xT[kb][:, sl],
                                     start=(kb == 0), stop=(kb == 15))
                nc.scalar.activation(out=qT[h][:, sl], in_=qps,
                                     func=mybir.ActivationFunctionType.Identity,
                                     bias=bq_sb[:, h:h + 1], scale=1.0)

            # VT projection for slice j, then PE-transpose to natural V
            vps = psum.tile([128, 512], F32, tag="acc", bufs=3, name=f"vps{j}")
            for kb in range(16):
                nc.tensor.matmul(out=vps, lhsT=wv_sb[kb], rhs=xT[kb][:, sl],
                                 start=(kb == 0), stop=(kb == 15))
            vt_sb = work.tile([128, 512], BF16, tag="vt", bufs=2,
                              name=f"vt{j}")
            nc.scalar.activation(out=vt_sb, in_=vps,
                                 func=mybir.ActivationFunctionType.Identity,
                                 bias=bv_sb[:, 0:1], scale=1.0)
            vtp = psum.tile([128, 512], BF16, tag="op", bufs=2, name=f"vtp{j}")
            for sub in range(4):
                nc.tensor.transpose(vtp[:, sub * 128:(sub + 1) * 128],
                                    vt_sb[:, sub * 128:(sub + 1) * 128],
                                    ident)
            for sub in range(4):
                nc.vector.tensor_copy(out=v_sb[4 * j + sub],
                                      in_=vtp[:, sub * 128:(sub + 1) * 128])

            # --- attention for all heads, tq-slice j ---
            ntk = 4 * (j + 1)
            for h in range(HPC):
                otps = psum.tile([128, 512], F32, tag="acc", bufs=3,
                                 name=f"otps{h}_{j}")
                racc = work.tile([128, 512], F32, tag="racc", bufs=2,
                                 name=f"racc{h}_{j}")
                for tkb in range(ntk):
                    sps = psum.tile([128, 512], F32, tag="sp", bufs=3,
                                    name=f"sps{h}_{j}_{tkb}")
                    nc.tensor.matmul(out=sps,
                                     lhsT=kT[:, tkb * 128:(tkb + 1) * 128],
                                     rhs=qT[h][:, sl],
                                     start=True, stop=True)
                    pt = work.tile([128, 512], BF16, tag="pt", bufs=6,
                                   name=f"pt{h}_{j}_{tkb}")
                    nc.scalar.activation(out=pt, in_=sps,
                                         func=mybir.ActivationFunctionType.Exp,
                                         scale=SCALE)
                    if tkb >= 4 * j:
                        # causal: zero pt where tq_free < tk_part + 128*r
                        nc.gpsimd.affine_select(
                            out=pt, in_=pt,
                            compare_op=mybir.AluOpType.is_ge,
                            fill=0.0,
                            base=-(128 * (tkb - 4 * j)),
                            pattern=[[1, 512]],
                            channel_multiplier=-1,
                        )
                    nc.tensor.matmul(out=otps, lhsT=v_sb[tkb], rhs=pt,
                                     start=(tkb == 0), stop=(tkb == ntk - 1))
                    if tkb == 0:
                        nc.vector.tensor_copy(out=racc, in_=pt)
                    else:
                        nc.vector.tensor_add(out=racc, in0=racc, in1=pt)
                rsb = psum.tile([128, 512], F32, tag="acc", bufs=3,
                                name=f"rsb{h}_{j}")
                nc.tensor.matmul(out=rsb, lhsT=ones32, rhs=racc,
                                 start=True, stop=True)
                rinv = work.tile([128, 512], F32, tag="rinv", bufs=2,
                                 name=f"rinv{h}_{j}")
                nc.vector.reciprocal_approx_fast(rinv, rsb)
                nc.vector.tensor_mul(out=oT[h][:, sl], in0=otps, in1=rinv)

            # --- output projection for the 4 t-tiles of slice j ---
            for sub in range(4):
                tt = 4 * j + sub
                ostg = work.tile([128, D], F32, tag="ostg", bufs=2,
                                 name=f"ostg{tt}")
                for n in range(4):
                    ops = psum.tile([128, 512], F32, tag="op", bufs=2,
                                    name=f"ops{tt}_{n}")
                    for h in range(HPC):
                        nc.tensor.matmul(
                            out=ops,
                            lhsT=oT[h][:, tt * 128:(tt + 1) * 128],
                            rhs=wo_sb[h][:, n * 512:(n + 1) * 512],
                            start=(h == 0), stop=(h == HPC - 1))
                    nc.vector.tensor_copy(out=ostg[:, n * 512:(n + 1) * 512],
                                          in_=ops)
                nc.sync.dma_start(out=part[tt * 128:(tt + 1) * 128, :],
                                  in_=ostg)

    nc.compile()
    return nc


def _get_nc():
    if "nc" not in _CACHE:
        _CACHE["nc"] = _build_nc()
    return _CACHE["nc"]


def _bf16(a):
    return np.ascontiguousarray(a.astype(ml_dtypes.bfloat16))


def kernel(x, Wq, bq, Wk, bk, Wv, bv, Wo, bo, **kw):
    x = np.asarray(x, dtype=np.float32)
    Wq = np.asarray(Wq, dtype=np.float32)
    Wk = np.asarray(Wk, dtype=np.float32)
    Wv = np.asarray(Wv, dtype=np.float32)
    Wo = np.asarray(Wo, dtype=np.float32)
    bq = np.asarray(bq, dtype=np.float32)
    bk = np.asarray(bk, dtype=np.float32)
    bv = np.asarray(bv, dtype=np.float32)
    bo = np.asarray(bo, dtype=np.float32)

    nc = _get_nc()
    xt_b = [_bf16(x[b].T) for b in range(B)]
    in_maps = []
    for c in range(NCORES):
        b = c // 4
        q = c % 4
        hs = q * HPC * DH          # column start in Wq / row start in Wo
        kv = q // 2
        bq_m = np.ascontiguousarray(
            bq[hs:hs + HPC * DH].reshape(HPC, DH).T)          # [128, 4]
        bk_m = np.ascontiguousarray(
            bk[kv * DH:(kv + 1) * DH].reshape(DH, 1))         # [128, 1]
        bv_m = np.ascontiguousarray(
            bv[kv * DH:(kv + 1) * DH].reshape(DH, 1))         # [128, 1]
        in_maps.append({
            "xt": xt_b[b],
            "wq": _bf16(Wq[:, hs:hs + HPC * DH]),
            "wk": _bf16(Wk[:, kv * DH:(kv + 1) * DH]),
            "wv": _bf16(Wv[:, kv * DH:(kv + 1) * DH]),
            "wo": _bf16(Wo[hs:hs + HPC * DH, :]),
            "bqm": bq_m,
            "bkm": bk_m,
            "bvm": bv_m,
        })

    res = run_bass_kernel_spmd(nc, in_maps, list(range(NCORES)),
                               **kw.get("_run_kwargs", {}))
    if kw.get("_return_res"):
        return res
    parts = [res.results[c]["part"] for c in range(NCORES)]
    out = np.empty((B, T, D), dtype=np.float32)
    for b in range(B):
        acc = parts[4 * b].astype(np.float32).copy()
        for q in range(1, 4):
            acc += parts[4 * b + q]
        out[b] = acc + bo[None, :]
    return out
